# revision 7
# baseline (speedup 1.0000x reference)
"""Bass/Trainium2 kernel for nn_DependencyParser (8 NeuronCores, SPMD).

kernel(**inputs) takes the FULL inputs of reference.setup_inputs() and
returns (arc_scores [512,512], label_scores [50,512], char_embeds [20,512]).

Sharding: all 8 cores run one SPMD program; per-core input data assigns
roles. Even cores run the forward LSTM recurrence, odd cores the backward
(on reversed inputs). Char-LSTM is word-sharded (64 words/core). One
AllReduce assembles lstm_out.T; the biaffine label tensor is sharded over
labels (7/core) and assembled with one AllGather.
"""
import os
import sys

sys.path.insert(0, "/opt/trn_rl_repo")

import numpy as np
import concourse.bass as bass
import concourse.bacc as bacc
import concourse.tile as tile
from concourse import mybir
from concourse import bass_utils
from concourse.bass import ds
from concourse.masks import make_identity

F32 = mybir.dt.float32
F16 = mybir.dt.float16
I32 = mybir.dt.int32
AF = mybir.ActivationFunctionType
ALU = mybir.AluOpType
PE = mybir.EngineType.PE

S = 512          # sentence length
WD, TD = 300, 100
CE, CH, MAXW = 20, 20, 16
LAB = 50
NCORE = 8
WPC = S // NCORE         # words per core for char lstm
LPC = 7                  # label slots per core (7*8=56 >= 50)
UNROLL = 16

_CACHE = {}


def _build(reps):
    nc = bacc.Bacc("TRN2", target_bir_lowering=False, debug=False,
                   num_devices=NCORE)

    def I(name, shape, dt=F32):
        return nc.dram_tensor(name, shape, dt, kind="ExternalInput").ap()

    word_emb = I("word_emb", [50000, WD])
    tag_emb = I("tag_emb", [50, TD])
    sent_idx = I("sent_idx", [128, 4], I32)
    tag_idx = I("tag_idx", [128, 4], I32)
    wihT = I("wihT", [640, 2048])            # [k-pad, gates] incl bias row 512
    whhT16 = I("whhT16", [512, 2048], F16)
    chars_f = I("chars_f", [1, WPC * MAXW])  # this core's chars as f32
    cmask = I("cmask", [1, WPC * MAXW])      # (t < L) mask
    cemb = I("cemb", [100, CE])
    cwihT = I("cwihT", [CE, 4 * CH])
    cwhhT = I("cwhhT", [CH, 4 * CH])
    cb_pad = I("cb_pad", [128, 1])
    attWT = I("attWT", [CH, 1])
    attb = I("attb", [CH, 1])
    selmask = I("selmask", [128, 2])
    best_f = I("best_f", [1, S])
    mlpWT = I("mlpWT", [4, 1152, 512])       # ldep, lhead, adep, ahead (aug)
    arcWT = I("arcWT", [640, 512])
    wlT = I("wlT", [LPC, 640, 512])
    wbias = I("wbias", [640, LPC])

    arc_out = nc.dram_tensor("arc_out", [S, S], F32, kind="ExternalOutput").ap()
    label_out = nc.dram_tensor("label_out", [S, LAB], F32,
                               kind="ExternalOutput").ap()
    char_out = nc.dram_tensor("char_out", [CH, WPC], F32,
                              kind="ExternalOutput").ap()

    dbg_names = []
    dbg_aps = {}
    if os.environ.get("KERNEL_DEBUG"):
        for nm, shape in [("d_zx", [128, 16 * S]), ("d_hstore", [128, 4 * S]),
                          ("d_lstmT", [128, 8 * S]), ("d_chars", [CH, WPC * MAXW]),
                          ("d_ladT", [128, 4 * S]), ("d_lh", [128, 4 * S]),
                          ("d_hselT", [128, 4 * S]), ("d_sel", [LPC, S]),
                          ("d_selall", [56, S]), ("d_xt", [128, 5 * S])]:
            dbg_aps[nm] = nc.dram_tensor(nm, shape, F32,
                                         kind="ExternalOutput").ap()
            dbg_names.append(nm)

    cc_in = nc.dram_tensor("cc_in", [1024, S], F32, kind="Internal").ap()
    cc_out = nc.dram_tensor("cc_out", [1024, S], F32, kind="Internal",
                            addr_space="Shared").ap()
    ag_in = nc.dram_tensor("ag_in", [LPC, S], F32, kind="Internal").ap()
    ag_out = nc.dram_tensor("ag_out", [NCORE * LPC, S], F32, kind="Internal",
                            addr_space="Shared").ap()
    rg = [list(range(NCORE))]

    with tile.TileContext(nc) as tc:
      with tc.tile_pool(name="keep", bufs=1) as keep, \
           tc.tile_pool(name="work", bufs=3) as work:

        ident = keep.tile([128, 128], F32)
        make_identity(nc, ident)
        lstmT = keep.tile([128, 9, S], F32)   # lstm_out.T chunks + ones chunk
        charres = keep.tile([CH, WPC], F32)

        # =========== PHASE A ===========
        with tc.tile_pool(name="pa", bufs=1) as pa:
          with tc.tile_pool(name="psA", bufs=2, space="PSUM") as psA, \
               tc.tile_pool(name="psT", bufs=1, space="PSUM") as psT, \
               tc.tile_pool(name="psC", bufs=1, space="PSUM") as psC:

            # ---- embedding gathers ----
            sidx = pa.tile([128, 4], I32)
            nc.sync.dma_start(out=sidx[:], in_=sent_idx)
            tidx = pa.tile([128, 4], I32)
            nc.sync.dma_start(out=tidx[:], in_=tag_idx)
            gw = pa.tile([128, 4, 384], F32)
            nc.vector.memset(gw[:], 0.0)
            gt = pa.tile([128, 4, 128], F32)
            nc.vector.memset(gt[:], 0.0)
            for c in range(4):
                nc.gpsimd.indirect_dma_start(
                    out=gw[:, c, 0:WD], out_offset=None, in_=word_emb,
                    in_offset=bass.IndirectOffsetOnAxis(ap=sidx[:, c:c+1], axis=0))
                nc.gpsimd.indirect_dma_start(
                    out=gt[:, c, 0:TD], out_offset=None, in_=tag_emb,
                    in_offset=bass.IndirectOffsetOnAxis(ap=tidx[:, c:c+1], axis=0))

            # ---- transpose to XT [128, 5, 512] ----
            XT = pa.tile([128, 5, S], F32)
            nc.vector.memset(XT[:, 4, :], 0.0)
            nc.vector.memset(XT[0:1, 4, :], 1.0)      # ones row (bias)
            for c in range(4):
                for d in range(3):
                    tp = psT.tile([128, 128], F32, tag="tp")
                    nc.tensor.transpose(out=tp[:], in_=gw[:, c, 128*d:128*d+128],
                                        identity=ident[:])
                    nc.vector.tensor_copy(out=XT[:, d, 128*c:128*c+128], in_=tp[:])
                tp = psT.tile([128, 128], F32, tag="tp")
                nc.tensor.transpose(out=tp[:], in_=gt[:, c, :], identity=ident[:])
                nc.vector.tensor_copy(out=XT[:, 3, 128*c:128*c+128], in_=tp[:])
            if dbg_aps:
                nc.sync.dma_start(out=dbg_aps["d_xt"],
                                  in_=XT[:].rearrange("p c t -> p (c t)"))

            # ---- zx = wihT.T @ XT -> [128, 512 t, 16 j] ----
            wih_sb = pa.tile([128, 5, 2048], F32)
            nc.sync.dma_start(out=wih_sb[:],
                              in_=wihT.rearrange("(k p) m -> p k m", p=128))
            zx = pa.tile([128, S, 16], F32)
            for j in range(16):
                pz = psA.tile([128, S], F32, tag="pz")
                for k in range(5):
                    nc.tensor.matmul(out=pz[:], lhsT=wih_sb[:, k, 128*j:128*j+128],
                                     rhs=XT[:, k, :], start=(k == 0), stop=(k == 4))
                nc.vector.tensor_copy(out=zx[:, :, j], in_=pz[:])
            if dbg_aps:
                nc.sync.dma_start(out=dbg_aps["d_zx"],
                                  in_=zx[:].rearrange("p t j -> p (t j)"))

            # ---- char pipeline ----
            NCH = WPC * MAXW       # 1024
            crep = pa.tile([128, NCH], F32)
            nc.sync.dma_start(out=crep[:], in_=bass.AP(
                tensor=chars_f.tensor, offset=chars_f.offset,
                ap=[[0, 128], chars_f.ap[1]]))
            io_i = pa.tile([128, 1], I32)
            nc.gpsimd.iota(io_i[:], pattern=[[0, 1]], base=0, channel_multiplier=1)
            io_f = pa.tile([128, 1], F32)
            nc.vector.tensor_copy(out=io_f[:], in_=io_i[:])
            onehot = pa.tile([128, NCH], F32)
            nc.vector.tensor_scalar(out=onehot[:], in0=crep[:], scalar1=io_f[:, 0:1],
                                    scalar2=None, op0=ALU.is_equal)

            cemb_sb = pa.tile([128, CE], F32)
            nc.vector.memset(cemb_sb[:], 0.0)
            nc.sync.dma_start(out=cemb_sb[0:100, :], in_=cemb)
            tpc = psC.tile([CE, 128], F32, tag="c1")
            nc.tensor.transpose(out=tpc[:], in_=cemb_sb[:], identity=ident[:])
            cembT = pa.tile([CE, 128], F32)
            nc.vector.tensor_copy(out=cembT[:], in_=tpc[:])
            cwih_sb = pa.tile([CE, 4 * CH], F32)
            nc.sync.dma_start(out=cwih_sb[:], in_=cwihT)
            pfu = psC.tile([128, 4 * CH], F32, tag="c1")
            nc.tensor.matmul(out=pfu[:], lhsT=cembT[:], rhs=cwih_sb[:],
                             start=True, stop=True)
            fused = pa.tile([128, 4 * CH], F32)
            nc.vector.tensor_copy(out=fused[:], in_=pfu[:])
            cwhh_sb = pa.tile([CH, 4 * CH], F32)
            nc.sync.dma_start(out=cwhh_sb[:], in_=cwhhT)
            cbp = pa.tile([128, 1], F32)
            nc.sync.dma_start(out=cbp[:], in_=cb_pad)

            hc = pa.tile([CH, WPC], F32)
            cc = pa.tile([CH, WPC], F32)
            nc.vector.memset(hc[:], 0.0)
            nc.vector.memset(cc[:], 0.0)
            hs_store = pa.tile([CH, NCH], F32)
            oh_ap = onehot[:]
            hsa = hs_store[:]
            for t in range(MAXW):
                pcz = psC.tile([128, WPC], F32, tag="cz")
                oh_t = bass.AP(tensor=oh_ap.tensor, offset=oh_ap.offset + t,
                               ap=[oh_ap.ap[0], [MAXW, WPC]])
                for g in range(4):
                    nc.tensor.matmul(out=pcz[32*g:32*g+CH, :],
                                     lhsT=fused[:, CH*g:CH*g+CH], rhs=oh_t,
                                     start=True, stop=False,
                                     tile_position=(0, 32 * g))
                    nc.tensor.matmul(out=pcz[32*g:32*g+CH, :],
                                     lhsT=cwhh_sb[:, CH*g:CH*g+CH], rhs=hc[:],
                                     start=False, stop=True,
                                     tile_position=(0, 32 * g))
                ga = work.tile([CH, 4, WPC], F32, tag="ga")
                for g, fn in ((0, AF.Sigmoid), (1, AF.Sigmoid),
                              (2, AF.Tanh), (3, AF.Sigmoid)):
                    nc.scalar.activation(out=ga[:, g, :],
                                         in_=pcz[32*g:32*g+CH, :], func=fn,
                                         bias=cbp[32*g:32*g+CH, 0:1])
                fcc = work.tile([CH, WPC], F32, tag="fcc")
                nc.vector.tensor_tensor(out=fcc[:], in0=ga[:, 1, :], in1=cc[:],
                                        op=ALU.mult)
                igc = work.tile([CH, WPC], F32, tag="igc")
                nc.vector.tensor_tensor(out=igc[:], in0=ga[:, 0, :], in1=ga[:, 2, :],
                                        op=ALU.mult)
                nc.vector.tensor_tensor(out=cc[:], in0=fcc[:], in1=igc[:],
                                        op=ALU.add)
                tcc = work.tile([CH, WPC], F32, tag="tcc")
                nc.scalar.activation(out=tcc[:], in_=cc[:], func=AF.Tanh)
                nc.vector.tensor_tensor(out=hc[:], in0=ga[:, 3, :], in1=tcc[:],
                                        op=ALU.mult)
                hdst = bass.AP(tensor=hsa.tensor, offset=hsa.offset + t,
                               ap=[hsa.ap[0], [MAXW, WPC]])
                nc.vector.tensor_copy(out=hdst, in_=hc[:])

            cmr = pa.tile([128, NCH], F32, tag="onehot")
            nc.sync.dma_start(out=cmr[0:CH, :], in_=bass.AP(
                tensor=cmask.tensor, offset=cmask.offset,
                ap=[[0, CH], cmask.ap[1]]))
            nc.vector.tensor_tensor(out=hs_store[:], in0=hs_store[:],
                                    in1=cmr[0:CH, :], op=ALU.mult)
            if dbg_aps:
                nc.sync.dma_start(out=dbg_aps["d_chars"], in_=hs_store[:])
            attw_sb = pa.tile([CH, 1], F32)
            nc.sync.dma_start(out=attw_sb[:], in_=attWT)
            ones_k1 = pa.tile([1, CH], F32)
            nc.vector.memset(ones_k1[:], 1.0)
            attb_sb = pa.tile([CH, 1], F32)
            nc.sync.dma_start(out=attb_sb[:], in_=attb)
            s_sb = pa.tile([1, NCH], F32)
            for hf in range(NCH // 512):
                sl = slice(512 * hf, 512 * hf + 512)
                ps_a = psC.tile([1, 512], F32, tag="ps_a")
                nc.tensor.matmul(out=ps_a[:], lhsT=attw_sb[:], rhs=hs_store[:, sl],
                                 start=True, stop=True)
                nc.vector.tensor_copy(out=s_sb[:, sl], in_=ps_a[:])
                ps_b = psC.tile([CH, 512], F32, tag="ps_b")
                nc.tensor.matmul(out=ps_b[:], lhsT=ones_k1[:], rhs=s_sb[:, sl],
                                 start=True, stop=True)
                nc.vector.tensor_tensor(out=hs_store[:, sl], in0=hs_store[:, sl],
                                        in1=ps_b[:], op=ALU.mult)
            racc = pa.tile([CH, WPC], F32)
            nc.vector.tensor_reduce(
                out=racc[:], in_=hs_store[:].rearrange("p (w t) -> p w t", t=MAXW),
                axis=mybir.AxisListType.X, op=ALU.add)
            nc.vector.tensor_scalar(out=charres[:], in0=racc[:],
                                    scalar1=attb_sb[:, 0:1], scalar2=None,
                                    op0=ALU.add)
            nc.sync.dma_start(out=char_out, in_=charres[:])

          # ---- recurrence weights + state (phase A PSUM pools closed) ----
          w_sb = keep.tile([128, 4, 2048], F16)
          nc.sync.dma_start(out=w_sb[:],
                            in_=whhT16.rearrange("(k p) m -> p k m", p=128))
          hstore = keep.tile([128, 4, S], F32)
          h16 = keep.tile([128, 4], F16)
          cst = keep.tile([128, 4], F32)
          nc.vector.memset(h16[:], 0.0)
          nc.vector.memset(cst[:], 0.0)

          # =========== RECURRENCE ===========
          with tc.tile_pool(name="psR", bufs=2, space="PSUM") as psR:
            def step(tex):
                acts = {}
                for gi, G in enumerate(("i", "f", "g", "o")):
                    psg = psR.tile([128, 4], F32, tag=f"ps_{G}")
                    for jl in range(4):
                        for k in range(4):
                            nc.tensor.matmul(
                                out=psg[:, jl:jl+1],
                                lhsT=w_sb[:, k, gi*512 + 128*jl: gi*512 + 128*jl + 128],
                                rhs=h16[:, k:k+1],
                                start=(k == 0), stop=(k == 3))
                    pre = work.tile([128, 4], F32, tag=f"pre_{G}")
                    nc.vector.tensor_tensor(
                        out=pre[:], in0=psg[:],
                        in1=zx[:, ds(tex, 1), 4*gi:4*gi+4], op=ALU.add)
                    a = work.tile([128, 4], F32, tag=f"a_{G}")
                    fn = AF.Tanh if G == "g" else AF.Sigmoid
                    nc.scalar.activation(out=a[:], in_=pre[:], func=fn)
                    acts[G] = a
                fc = work.tile([128, 4], F32, tag="fc")
                nc.vector.tensor_tensor(out=fc[:], in0=acts["f"][:], in1=cst[:],
                                        op=ALU.mult)
                ig = work.tile([128, 4], F32, tag="ig")
                nc.vector.tensor_tensor(out=ig[:], in0=acts["i"][:],
                                        in1=acts["g"][:], op=ALU.mult)
                nc.vector.tensor_tensor(out=cst[:], in0=fc[:], in1=ig[:],
                                        op=ALU.add)
                tct = work.tile([128, 4], F32, tag="tct")
                nc.scalar.activation(out=tct[:], in_=cst[:], func=AF.Tanh)
                nc.vector.tensor_tensor(out=h16[:], in0=acts["o"][:],
                                        in1=tct[:], op=ALU.mult)
                nc.vector.tensor_copy(out=hstore[:, :, ds(tex, 1)], in_=h16[:])

            with tc.For_i(0, reps, 1, name="repl") as _r:
                with tc.For_i(0, S, UNROLL, staggered_reset=True,
                              hint_engines=(PE,), name="stepl") as iv:
                    for s_ in range(UNROLL):
                        step(iv + s_)

          if dbg_aps:
              nc.sync.dma_start(out=dbg_aps["d_hstore"],
                                in_=hstore[:].rearrange("p c t -> p (c t)"))

          # =========== contribution ===========
          smk = pa.tile([128, 2], F32)
          nc.sync.dma_start(out=smk[:], in_=selmask)
          contrib = pa.tile([128, 8, S], F32)
          nc.vector.tensor_scalar(out=contrib[:, 0:4, :], in0=hstore[:],
                                  scalar1=smk[:, 0:1], scalar2=None, op0=ALU.mult)
          hsap = hstore[:]
          flip = bass.AP(tensor=hsap.tensor, offset=hsap.offset + (S - 1),
                         ap=[hsap.ap[0], [S, 4], [-1, S]])
          nc.vector.tensor_scalar(out=contrib[:, 4:8, :], in0=flip,
                                  scalar1=smk[:, 1:2], scalar2=None, op0=ALU.mult)
          nc.sync.dma_start(out=cc_in.rearrange("(c p) t -> p c t", p=128),
                            in_=contrib[:])

        # phase A SBUF pool released here
        nc.gpsimd.collective_compute("AllReduce", ALU.add, ins=[cc_in],
                                     outs=[cc_out], replica_groups=rg)

        # =========== PHASE D ===========
        with tc.tile_pool(name="pd", bufs=1) as pd, \
             tc.tile_pool(name="pd2", bufs=2) as pd2, \
             tc.tile_pool(name="psD", bufs=4, space="PSUM") as psD, \
             tc.tile_pool(name="psDs", bufs=2, space="PSUM") as psDs:

            nc.sync.dma_start(out=lstmT[:, 0:8, :],
                              in_=cc_out.rearrange("(c p) t -> p c t", p=128))
            nc.vector.memset(lstmT[:, 8, :], 0.0)
            nc.vector.memset(lstmT[0:1, 8, :], 1.0)
            if dbg_aps:
                nc.sync.dma_start(out=dbg_aps["d_lstmT"],
                                  in_=lstmT[:, 0:8, :].rearrange("p c t -> p (c t)"))

            # ---- 4 MLPs ----
            DaT = pd.tile([128, 5, S], F32)
            nc.vector.memset(DaT[:, 4, :], 0.0)
            nc.vector.memset(DaT[0:1, 4, :], 1.0)      # dep ones row (j=512)
            ADt = pd.tile([128, 4, S], F32)
            AHt = pd.tile([128, 5, S], F32)
            nc.vector.memset(AHt[:, 4, :], 0.0)
            nc.vector.memset(AHt[0:1, 4, :], 1.0)      # head ones row
            LH = pd.tile([128, 4, S], F32)

            for mi, dest in ((0, DaT), (1, LH), (2, ADt), (3, AHt)):
                wmlp = pd2.tile([128, 9, 512], F32, tag="wmlp")
                nc.sync.dma_start(out=wmlp[:],
                                  in_=mlpWT[mi].rearrange("(k p) m -> p k m", p=128))
                for m in range(4):
                    pm = psD.tile([128, 512], F32, tag="mm")
                    if mi == 1:   # label_head non-transposed
                        for k in range(9):
                            nc.tensor.matmul(out=pm[:],
                                             lhsT=lstmT[:, k, 128*m:128*m+128],
                                             rhs=wmlp[:, k, :],
                                             start=(k == 0), stop=(k == 8))
                    else:
                        for k in range(9):
                            nc.tensor.matmul(out=pm[:],
                                             lhsT=wmlp[:, k, 128*m:128*m+128],
                                             rhs=lstmT[:, k, :],
                                             start=(k == 0), stop=(k == 8))
                    nc.scalar.activation(out=dest[:, m, :], in_=pm[:], func=AF.Relu)
            if dbg_aps:
                nc.sync.dma_start(out=dbg_aps["d_ladT"],
                                  in_=DaT[:, 0:4, :].rearrange("p c t -> p (c t)"))
                nc.sync.dma_start(out=dbg_aps["d_lh"],
                                  in_=LH[:].rearrange("p c t -> p (c t)"))

            # ---- P one-hot [t', t] + HselT ----
            bestr = pd.tile([128, S], F32)
            nc.sync.dma_start(out=bestr[:], in_=bass.AP(
                tensor=best_f.tensor, offset=best_f.offset,
                ap=[[0, 128], best_f.ap[1]]))
            Pm = pd.tile([128, 4, S], F32)
            for c in range(4):
                ii = pd.tile([128, 1], I32, tag="pio")
                nc.gpsimd.iota(ii[:], pattern=[[0, 1]], base=128 * c,
                               channel_multiplier=1)
                iif = pd.tile([128, 1], F32, tag="piof")
                nc.vector.tensor_copy(out=iif[:], in_=ii[:])
                nc.vector.tensor_scalar(out=Pm[:, c, :], in0=bestr[:],
                                        scalar1=iif[:, 0:1], scalar2=None,
                                        op0=ALU.is_equal)
            HselT = pd.tile([128, 4, S], F32)
            for hm in range(4):
                ph = psD.tile([128, S], F32, tag="mm")
                for tc_ in range(4):
                    nc.tensor.matmul(out=ph[:], lhsT=LH[:, tc_, 128*hm:128*hm+128],
                                     rhs=Pm[:, tc_, :],
                                     start=(tc_ == 0), stop=(tc_ == 3))
                nc.vector.tensor_copy(out=HselT[:, hm, :], in_=ph[:])
            if dbg_aps:
                nc.sync.dma_start(out=dbg_aps["d_hselT"],
                                  in_=HselT[:].rearrange("p c t -> p (c t)"))

            # ---- per-label biaffine ----
            wb_sb = pd.tile([128, 5, LPC], F32)
            nc.sync.dma_start(out=wb_sb[:],
                              in_=wbias.rearrange("(k p) l -> p k l", p=128))
            ones128 = pd.tile([128, 1], F32)
            nc.vector.memset(ones128[:], 1.0)
            for li in range(LPC):
                wl = pd2.tile([128, 5, 512], F32, tag="wl")
                nc.sync.dma_start(out=wl[:],
                                  in_=wlT[li].rearrange("(k p) m -> p k m", p=128))
                psel = psDs.tile([1, S], F32, tag="sel")
                # bias term: sel += wbias[:, li].T @ DaT
                for k in range(5):
                    nc.tensor.matmul(out=psel[:], lhsT=wb_sb[:, k, li:li+1],
                                     rhs=DaT[:, k, :],
                                     start=(k == 0), stop=False)
                for hm in range(4):
                    pt1 = psD.tile([128, S], F32, tag="mm")
                    for k in range(5):
                        nc.tensor.matmul(out=pt1[:],
                                         lhsT=wl[:, k, 128*hm:128*hm+128],
                                         rhs=DaT[:, k, :],
                                         start=(k == 0), stop=(k == 4))
                    prod = work.tile([128, S], F32, tag="prod")
                    nc.vector.tensor_tensor(out=prod[:], in0=pt1[:],
                                            in1=HselT[:, hm, :], op=ALU.mult)
                    nc.tensor.matmul(out=psel[:], lhsT=ones128[:], rhs=prod[:],
                                     start=False, stop=(hm == 3))
                srow = work.tile([1, S], F32, tag="srow")
                nc.vector.tensor_copy(out=srow[:], in_=psel[:])
                nc.sync.dma_start(out=ag_in[li:li+1, :], in_=srow[:])
                if dbg_aps:
                    nc.sync.dma_start(out=dbg_aps["d_sel"][li:li+1, :], in_=srow[:])
            nc.gpsimd.collective_compute("AllGather", ALU.bypass, ins=[ag_in],
                                         outs=[ag_out], replica_groups=rg)
            selall = pd.tile([NCORE * LPC, S], F32)
            nc.sync.dma_start(out=selall[:], in_=ag_out)
            if dbg_aps:
                nc.sync.dma_start(out=dbg_aps["d_selall"], in_=selall[:])

            # ---- label log_softmax (over labels) ----
            for c in range(4):
                pt = psD.tile([128, 56], F32, tag="mm")
                nc.tensor.transpose(out=pt[:, 0:56],
                                    in_=selall[:, 128*c:128*c+128],
                                    identity=ident[0:56, 0:56])
                st = work.tile([128, LAB], F32, tag="st")
                nc.vector.tensor_copy(out=st[:], in_=pt[:, 0:LAB])
                mx = work.tile([128, 1], F32, tag="mx")
                nc.vector.tensor_reduce(out=mx[:], in_=st[:],
                                        axis=mybir.AxisListType.X, op=ALU.max,
                                        negate=True)
                ex = work.tile([128, LAB], F32, tag="ex")
                nc.scalar.activation(out=ex[:], in_=st[:], func=AF.Exp,
                                     bias=mx[:, 0:1])
                sm = work.tile([128, 1], F32, tag="sm")
                nc.vector.tensor_reduce(out=sm[:], in_=ex[:],
                                        axis=mybir.AxisListType.X, op=ALU.add)
                lg = work.tile([128, 1], F32, tag="lg")
                nc.scalar.activation(out=lg[:], in_=sm[:], func=AF.Ln)
                lout = work.tile([128, LAB], F32, tag="lout")
                nc.vector.tensor_scalar(out=lout[:], in0=st[:],
                                        scalar1=mx[:, 0:1], scalar2=lg[:, 0:1],
                                        op0=ALU.add, op1=ALU.subtract)
                nc.sync.dma_start(out=label_out[128*c:128*c+128, :], in_=lout[:])

            # ---- arc pipeline ----
            arcw = pd2.tile([128, 5, 512], F32, tag="wl")
            nc.sync.dma_start(out=arcw[:],
                              in_=arcWT.rearrange("(k p) m -> p k m", p=128))
            BT = pd.tile([128, 4, S], F32)
            for m in range(4):
                pbt = psD.tile([128, S], F32, tag="mm")
                for k in range(5):
                    nc.tensor.matmul(out=pbt[:], lhsT=arcw[:, k, 128*m:128*m+128],
                                     rhs=AHt[:, k, :], start=(k == 0), stop=(k == 4))
                nc.vector.tensor_copy(out=BT[:, m, :], in_=pbt[:])
            for m in range(4):
                pM = psD.tile([128, S], F32, tag="mm")
                for k in range(4):
                    nc.tensor.matmul(out=pM[:], lhsT=BT[:, k, 128*m:128*m+128],
                                     rhs=ADt[:, k, :], start=(k == 0), stop=(k == 3))
                mrow = work.tile([128, S], F32, tag="mrow")
                nc.vector.tensor_copy(out=mrow[:], in_=pM[:])
                mx = work.tile([128, 1], F32, tag="mx")
                nc.vector.tensor_reduce(out=mx[:], in_=mrow[:],
                                        axis=mybir.AxisListType.X, op=ALU.max,
                                        negate=True)
                ex = work.tile([128, S], F32, tag="exa")
                nc.scalar.activation(out=ex[:], in_=mrow[:], func=AF.Exp,
                                     bias=mx[:, 0:1])
                sm = work.tile([128, 1], F32, tag="sm")
                nc.vector.tensor_reduce(out=sm[:], in_=ex[:],
                                        axis=mybir.AxisListType.X, op=ALU.add)
                rcp = work.tile([128, 1], F32, tag="rcp")
                nc.vector.reciprocal(out=rcp[:], in_=sm[:])
                aout = work.tile([128, S], F32, tag="aout")
                nc.vector.tensor_scalar(out=aout[:], in0=ex[:],
                                        scalar1=rcp[:, 0:1], scalar2=None,
                                        op0=ALU.mult)
                nc.sync.dma_start(out=arc_out[128*m:128*m+128, :], in_=aout[:])

    nc.compile()
    return nc, dbg_names


def _prep_inputs(inputs):
    f32 = np.float32
    sentence = np.asarray(inputs["sentence"]).astype(np.int32)
    tags = np.asarray(inputs["tags"]).astype(np.int32)
    chars = np.asarray(inputs["chars"]).astype(np.int32)
    char_lengths = np.asarray(inputs["char_lengths"]).astype(np.int32)
    best_arcs = np.asarray(inputs["best_arcs"]).astype(np.int32)
    g = lambda k: np.asarray(inputs[k], dtype=f32)

    word_emb = g("word_emb"); tag_emb = g("tag_emb"); char_emb = g("char_emb")
    att_W = g("att_W"); att_b = g("att_b")
    cW_ih = g("cW_ih"); cW_hh = g("cW_hh"); cb = g("cb")
    biaff_arc_W = g("biaff_arc_W"); biaff_label_W = g("biaff_label_W")

    def mlp_aug(W, b):
        out = np.zeros((1152, 512), f32)
        out[0:1024] = W.T
        out[1024] = b
        return out

    mlpWT = np.stack([
        mlp_aug(g("label_dep_W"), g("label_dep_b")),
        mlp_aug(g("label_head_W"), g("label_head_b")),
        mlp_aug(g("arc_dep_W"), g("arc_dep_b")),
        mlp_aug(g("arc_head_W"), g("arc_head_b")),
    ])
    arcWT = np.zeros((640, 512), f32)
    arcWT[0:513] = biaff_arc_W.T

    cb_pad = np.zeros((128, 1), f32)
    for gi in range(4):
        cb_pad[32*gi:32*gi+CH, 0] = cb[CH*gi:CH*gi+CH]

    cmask_full = (np.arange(MAXW)[None, :] < char_lengths[:, None]).astype(f32)

    shared = dict(
        word_emb=word_emb, tag_emb=tag_emb, cemb=char_emb,
        cwihT=np.ascontiguousarray(cW_ih.T), cwhhT=np.ascontiguousarray(cW_hh.T),
        cb_pad=cb_pad,
        attWT=np.ascontiguousarray(att_W.T),
        attb=np.full((CH, 1), float(att_b[0]), f32),
        best_f=best_arcs.astype(f32).reshape(1, S),
        mlpWT=mlpWT, arcWT=arcWT,
    )

    in_maps = []
    for r in range(NCORE):
        fwd = (r % 2 == 0)
        sent = sentence if fwd else sentence[::-1]
        tg = tags if fwd else tags[::-1]
        W_ih = g("W_ih_f") if fwd else g("W_ih_b")
        W_hh = g("W_hh_f") if fwd else g("W_hh_b")
        b = g("b_f") if fwd else g("b_b")
        wihT_ = np.zeros((640, 2048), f32)
        wihT_[0:300] = W_ih[:, 0:300].T
        wihT_[384:484] = W_ih[:, 300:400].T
        wihT_[512] = b
        whhT16_ = np.ascontiguousarray(W_hh.T).astype(np.float16)

        wl = np.zeros((LPC, 640, 512), f32)
        wb = np.zeros((640, LPC), f32)
        for i in range(LPC):
            l = r * LPC + i
            if l < LAB:
                wl[i, 0:513, 0:512] = biaff_label_W[l, 0:512, 0:513].T
                wb[0:513, i] = biaff_label_W[l, 512, :]
        selm = np.zeros((128, 2), f32)
        selm[:, 0] = 0.25 if fwd else 0.0
        selm[:, 1] = 0.0 if fwd else 0.25

        cw = chars[r*WPC:(r+1)*WPC]
        cm = cmask_full[r*WPC:(r+1)*WPC]
        in_maps.append(dict(
            shared,
            sent_idx=np.ascontiguousarray(sent.reshape(4, 128).T),
            tag_idx=np.ascontiguousarray(tg.reshape(4, 128).T),
            wihT=wihT_, whhT16=whhT16_,
            chars_f=cw.astype(f32).reshape(1, -1),
            cmask=np.ascontiguousarray(cm.reshape(1, -1)),
            selmask=selm, wlT=wl, wbias=wb,
        ))
    return in_maps


def kernel(_reps=1, **inputs):
    key = ("nc", _reps, bool(os.environ.get("KERNEL_DEBUG")))
    if key not in _CACHE:
        _CACHE[key] = _build(_reps)
    nc, dbg = _CACHE[key]
    in_maps = _prep_inputs(inputs)
    r = bass_utils.run_bass_kernel_spmd(nc, in_maps, core_ids=list(range(NCORE)))
    res0 = r.results[0]
    arc_scores = res0["arc_out"]
    label_scores = np.ascontiguousarray(res0["label_out"].T)
    char_embeds = np.concatenate(
        [r.results[c]["char_out"] for c in range(NCORE)], axis=1)
    if dbg:
        kernel._debug = r.results
    return (arc_scores, label_scores, char_embeds)
